# revision 2
# baseline (speedup 1.0000x reference)
"""Trainium2 Bass kernel for BA3MotifNet (4-layer LEConv GNN + mean-pool + MLP).

SPMD across 8 NeuronCores, single compiled graph; all per-core variation is
carried in the input data (index streams), never in instruction structure.

  - Nodes dst-sharded at graph boundaries (batch sorted): core c owns graphs
    [125c,125(c+1)) and their nodes, padded to NODE_PAD=12800/core.
  - Per layer: a = h@W1+b1 computed in transposed orientation, PE-transposed
    to node-major, DMA'd to DRAM, AllGather -> a_full [102400,64].
  - agg_i = sum_{e:dst=i} ew_e*a[src_e] - (h@W2)_i * degw_i.
    Gather: SWDGE dma_gather of 256B rows from a_full; int16 indices => four
    source quarters of 25600 rows; edges bucketed (quarter, window of 128
    dsts), tiled 128/tile, padded to a core-uniform tile table T[w,q].
    Scatter: PE matmul aggT[f,d] += gathered[e,f].T @ onehot[e,d] into a PSUM
    window [65,128]; onehot [128,128] bf16 built by gpsimd local_scatter.
    t3 = h@W3+b3 initializes each pass-0 psum window; deg_w rides psum row 64
    via a ones-column matmul per tile (layer 0 only).
  - h = relu(aggT + t2T*(-degw)); layer 3 also produces node-major bf16 h.
  - Mean-pool via (1/cnt)-valued one-hot matmuls; 2-layer MLP on-core.
  - Out: per-core [128,4] f32 -> host concat -> [1000,3].
"""

import os
import sys

import numpy as np

sys.path.insert(0, "/opt/trn_rl_repo")

ABL_GATHER = os.environ.get("ABL_GATHER", "0") == "1"   # replace dma_gather w/ memset
ABL_LSCAT = os.environ.get("ABL_LSCAT", "0") == "1"     # replace local_scatter w/ memset

FULL_CFG = dict(
    n_nodes=100000, n_edges=3200000, n_graphs=1000, hid=64, n_layers=4,
    nc=8, node_pad=12800, call_tiles=36, ls_t=14,
)


# --------------------------------------------------------------- host prep
def shard_and_pack(inputs, cfg):
    NC, NP = cfg["nc"], cfg["node_pad"]
    NW, NQ = NP // 128, 4
    QROWS = NP * NC // NQ
    G = cfg["n_graphs"]
    GPC = G // NC
    assert QROWS <= 32768

    x = np.asarray(inputs["x"], np.float32)
    ei = np.asarray(inputs["edge_index"], np.int64)
    ew = np.asarray(inputs["edge_attr"], np.float32)
    batch = np.asarray(inputs["batch"], np.int64)
    N = x.shape[0]
    NF = x.shape[1]

    gs = np.searchsorted(batch, np.arange(G + 1))
    nstart = gs[np.arange(NC + 1) * GPC]
    ncnt = np.diff(nstart)
    if ncnt.max() > NP:                                    # rare: grow pad
        NP = int(-(-int(ncnt.max()) // 512) * 512)
        cfg = dict(cfg, node_pad=NP)
        NW = NP // 128
        QROWS = NP * NC // NQ
        assert QROWS <= 32768

    shard_of = np.searchsorted(nstart[1:], np.arange(N), side="right")
    src, dst = ei[0], ei[1]
    e_core = shard_of[dst]
    # quarter of a src node depends only on its shard (QROWS == 2*NP)
    e_q = shard_of[src] * NP // QROWS

    # degree-balanced window packing per core: relabel local node ids so every
    # (window, quarter) edge count is as even as possible (pulls the uniform
    # tile table T[w,q] from 9 down to 8 -> ~10% less gather/PE work).
    newloc = np.zeros(N, np.int64)
    for c in range(NC):
        n_c = int(ncnt[c])
        deg4 = np.zeros((NP, NQ), np.int64)
        selc = e_core == c
        np.add.at(deg4, (dst[selc] - nstart[c], e_q[selc]), 1)
        deg4 = deg4[:n_c]
        order = np.argsort(-deg4.sum(1), kind="stable")
        loads = np.zeros((NW, NQ), np.int64)
        fill = np.zeros(NW, np.int64)
        assign = np.zeros(n_c, np.int64)
        for n in order:
            new_loads = loads + deg4[n]
            # hard-penalize crossing the 8-tile (1024-edge) boundary; keep
            # a small safety margin, tie-break on balance
            over = np.maximum(0, new_loads - 1016).sum(1)
            cand = over * 1e6 + new_loads.max(1).astype(np.float64)
            cand[fill >= 128] = np.inf
            wsel = int(np.argmin(cand))
            assign[n] = wsel * 128 + fill[wsel]
            fill[wsel] += 1
            loads[wsel] += deg4[n]
        newloc[nstart[c]: nstart[c] + n_c] = assign

    spad = shard_of * NP + newloc
    dstloc = newloc[dst]
    e_w = dstloc >> 7

    cnt = np.zeros((NC, NW, NQ), np.int64)
    np.add.at(cnt, (e_core, e_w, e_q), 1)
    T = np.maximum(1, -(-cnt.max(axis=0) // 128))          # [NW, NQ]

    ntiles_q = T.sum(axis=0)
    ntiles = int(ntiles_q.sum())
    LS_T = cfg["ls_t"]
    ntiles_pad = -(-ntiles // LS_T) * LS_T

    tile_w = np.concatenate([np.repeat(np.arange(NW), T[:, q]) for q in range(NQ)])
    qstart_tiles = np.concatenate([[0], np.cumsum(ntiles_q)]).astype(np.int64)

    per_core = []
    for c in range(NC):
        sel = e_core == c
        s_qi = (spad[src[sel]] % QROWS).astype(np.int64)
        s_q, s_w = e_q[sel], e_w[sel]
        s_off = (dstloc[sel] & 127).astype(np.int64)
        s_ew = ew[sel]

        order = np.lexsort((s_off, s_w, s_q))
        s_qi, s_q, s_w, s_off, s_ew = (a[order] for a in (s_qi, s_q, s_w, s_off, s_ew))
        blk_sizes = (T.T.reshape(-1) * 128)
        blk_base = np.concatenate([[0], np.cumsum(blk_sizes)])[:-1].reshape(NQ, NW)
        key = s_q * NW + s_w
        grp_start = np.searchsorted(key, np.arange(NQ * NW), side="left")
        slot = blk_base[s_q, s_w] + (np.arange(key.size) - grp_start[key])

        nslots = ntiles * 128
        gidx = np.zeros(nslots, np.int16)
        ewv = np.zeros(nslots, np.float32)
        offv = np.full(nslots, -1, np.int64)
        gidx[slot] = s_qi.astype(np.int16)
        ewv[slot] = s_ew
        offv[slot] = s_off

        gw = np.tile(gidx.reshape(-1, 16).T, (8, 1))       # [128, nslots/16]

        offm = offv.reshape(ntiles, 128).T
        ewm = ewv.reshape(ntiles, 128).T
        tmod = np.arange(ntiles) % LS_T
        sidx = np.where(offm >= 0, tmod[None, :] * 128 + offm, -1).astype(np.int16)
        sidx = np.pad(sidx, ((0, 0), (0, ntiles_pad - ntiles)), constant_values=-1)
        ewm = np.pad(ewm, ((0, 0), (0, ntiles_pad - ntiles)))

        loc = newloc[nstart[c]: nstart[c + 1]]
        xT1 = np.zeros((NF + 1, NP), np.float32)
        xT1[:NF, loc] = x[nstart[c]: nstart[c + 1]].T
        xT1[NF, :] = 1.0

        nb = (batch[nstart[c]: nstart[c + 1]] - c * GPC).astype(np.int64)
        cnts = np.bincount(nb, minlength=GPC).astype(np.float32)
        pool = np.zeros((128, NP), np.float32)
        pool[loc & 127, (loc >> 7) * 128 + nb] = 1.0 / np.maximum(cnts[nb], 1.0)

        per_core.append(dict(gidx=gw, sidx=sidx, ew=ewm, xT1=xT1, pool=pool,
                             ngraphs=GPC))

    meta = dict(T=T, ntiles_q=ntiles_q.astype(int), qstart=qstart_tiles,
                tile_w=tile_w.astype(int), ntiles=ntiles, ntiles_pad=ntiles_pad,
                NW=NW, NQ=NQ, QROWS=QROWS, NF=NF, cfg=cfg)
    return per_core, meta


def weights_map(inputs):
    f32 = np.float32
    vs = np.vstack
    w = {"embWb": vs([np.asarray(inputs["emb_w"], f32),
                      np.asarray(inputs["emb_b"], f32)[None]]),
         "L1b": vs([np.asarray(inputs["lin1_w"], f32),
                    np.asarray(inputs["lin1_b"], f32)[None]]),
         "L2b": vs([np.asarray(inputs["lin2_w"], f32),
                    np.asarray(inputs["lin2_b"], f32)[None]]),
         "ident": np.eye(128, dtype=f32)}
    L = np.asarray(inputs["conv_w1"]).shape[0]
    for l in range(L):
        w[f"W1b_{l}"] = vs([np.asarray(inputs["conv_w1"][l], f32),
                            np.asarray(inputs["conv_b1"][l], f32)[None]])
        w[f"W2_{l}"] = np.asarray(inputs["conv_w2"][l], f32)
        w[f"W3b_{l}"] = vs([np.asarray(inputs["conv_w3"][l], f32),
                            np.asarray(inputs["conv_b3"][l], f32)[None]])
    return w


# --------------------------------------------------------------- builder
def build_graph(meta):
    from concourse import bacc, mybir, tile

    cfg = meta["cfg"]
    NC, H, L = cfg["nc"], cfg["hid"], cfg["n_layers"]
    NP, NW, NQ, QROWS = cfg["node_pad"], meta["NW"], meta["NQ"], meta["QROWS"]
    NF = meta["NF"]
    H2 = 2 * H                                  # padded bf16 a-row (256B)
    ntiles, ntiles_pad = meta["ntiles"], meta["ntiles_pad"]
    tile_w, qstart, ntiles_q = meta["tile_w"], meta["qstart"], meta["ntiles_q"]
    CALL_T, LS_T = cfg["call_tiles"], cfg["ls_t"]
    f32, bf16, i16 = mybir.dt.float32, mybir.dt.bfloat16, mybir.dt.int16
    AF = mybir.ActivationFunctionType
    NCH = NP // 512

    nc = bacc.Bacc(num_devices=NC)

    gidx_d = nc.declare_dram_parameter("gidx", [128, ntiles * 8], i16, False)
    sidx_d = nc.declare_dram_parameter("sidx", [128, ntiles_pad], i16, False)
    ew_d = nc.declare_dram_parameter("ew", [128, ntiles_pad], bf16, False)
    xT1_d = nc.declare_dram_parameter("xT1", [NF + 1, NP], bf16, False)
    pool_d = nc.declare_dram_parameter("pool", [128, NP], bf16, False)
    wnames = (["embWb", "L1b", "L2b", "ident"]
              + [f"{p}_{l}" for l in range(L) for p in ("W1b", "W2", "W3b")])
    wshape = {"embWb": [NF + 1, H], "L1b": [H + 1, H], "L2b": [H + 1, 3],
              "ident": [128, 128]}
    wdt = {"embWb": bf16, "L1b": f32, "L2b": f32, "ident": f32}
    for l in range(L):
        wshape[f"W1b_{l}"] = [H + 1, H]
        wshape[f"W2_{l}"] = [H, H]
        wshape[f"W3b_{l}"] = [H + 1, H]
        wdt[f"W1b_{l}"] = wdt[f"W2_{l}"] = wdt[f"W3b_{l}"] = bf16
    wd = {k: nc.declare_dram_parameter(k, wshape[k], wdt[k], False)
          for k in wnames}
    out_d = nc.declare_dram_parameter("out", [128, 4], f32, True)

    with tile.TileContext(nc) as tc:
        with (
            tc.tile_pool(name="res", bufs=1) as res,
            tc.tile_pool(name="dram", bufs=1, space="DRAM") as dram,
            tc.tile_pool(name="stage", bufs=2) as stage,
            tc.tile_pool(name="gbuf", bufs=2) as gpool,
            tc.tile_pool(name="ohbuf", bufs=3) as ohpool,
            tc.tile_pool(name="ixbuf", bufs=3) as ixpool,
            tc.tile_pool(name="scr", bufs=3) as scr,
            tc.tile_pool(name="psA", bufs=2, space="PSUM") as psA,
            tc.tile_pool(name="psB", bufs=2, space="PSUM") as psB,
            tc.tile_pool(name="psC", bufs=2, space="PSUM") as psC,
            tc.tile_pool(name="psD", bufs=2, space="PSUM") as psD,
        ):
            a_loc = dram.tile([NP, H2], bf16)
            a_full = dram.tile([NP * NC, H2], bf16)

            hT = res.tile([H + 1, NP], bf16, tag="hT")
            t2T = res.tile([H, NP], bf16, tag="t2T")
            aggT = res.tile([H + 1, NP], f32, tag="aggT")
            dgw = res.tile([H, NP], bf16, tag="dgw")
            sidx_s = res.tile([128, ntiles_pad], i16, tag="sidx")
            ew_s = res.tile([128, ntiles_pad], bf16, tag="ew")
            h_nm = res.tile([128, NW * H], bf16, tag="h_nm")
            ones_col = res.tile([128, 1], bf16, tag="ones")
            ones_row = res.tile([1, H], f32, tag="ones_row")
            wts = {k: res.tile(wshape[k], wdt[k], tag=k, name=k) for k in wnames}
            ident = wts["ident"]

            nc.vector.memset(ones_col[:], 1.0)
            nc.vector.memset(ones_row[:], 1.0)
            for k in wnames:
                nc.sync.dma_start(wts[k][:], wd[k][:])
            nc.sync.dma_start(sidx_s[:], sidx_d[:])
            nc.sync.dma_start(ew_s[:], ew_d[:])

            # h0T = embWb.T @ xT1 (streamed)
            for ck in range(NCH):
                sl = slice(ck * 512, (ck + 1) * 512)
                xc = stage.tile([NF + 1, 512], bf16, tag="xc")
                nc.sync.dma_start(xc[:], xT1_d[:, sl])
                ps = psA.tile([H, 512], f32, tag="dps")
                nc.tensor.matmul(ps[:], wts["embWb"][:], xc[:],
                                 start=True, stop=True)
                nc.scalar.activation(hT[:H, sl], ps[:], AF.Copy)
            nc.vector.memset(hT[H:H + 1, :], 1.0)

            for l in range(L):
                layer0 = l == 0
                # ---- dense: a -> a_loc (node-major bf16, 128-padded), t2T
                for ck in range(NCH):
                    sl = slice(ck * 512, (ck + 1) * 512)
                    ps = psA.tile([H, 512], f32, tag="dps")
                    nc.tensor.matmul(ps[:], wts[f"W1b_{l}"][:], hT[:, sl],
                                     start=True, stop=True)
                    aTc = stage.tile([H, 512], f32, tag="aTc")
                    nc.scalar.activation(aTc[:], ps[:], AF.Copy)
                    ast = stage.tile([128, 4, H2], bf16, tag="ast")
                    nc.vector.memset(ast[:, :, H:], 0.0)
                    for j in range(4):
                        pst = psB.tile([128, H], f32, tag="tps")
                        nc.tensor.transpose(pst[:], aTc[:, j * 128:(j + 1) * 128],
                                            ident[:H, :H])
                        nc.vector.tensor_copy(ast[:, j, :H], pst[:])
                    nc.sync.dma_start(
                        a_loc[sl, :].rearrange("(j p) f -> p j f", p=128), ast[:])
                    ps2 = psA.tile([H, 512], f32, tag="dps")
                    nc.tensor.matmul(ps2[:], wts[f"W2_{l}"][:], hT[:H, sl],
                                     start=True, stop=True)
                    nc.scalar.activation(t2T[:, sl], ps2[:], AF.Copy)

                nc.gpsimd.collective_compute(
                    "AllGather", mybir.AluOpType.bypass,
                    replica_groups=[list(range(NC))],
                    ins=[a_loc[:].opt()], outs=[a_full[:].opt()])

                # ---- gather + scatter: walk global tile stream
                ohb = None
                gb = None
                cur_w, cur_q = -1, -1
                ps = None
                dps = None
                first_in_win = False

                def flush_window(q):
                    wsl = slice(cur_w * 128, (cur_w + 1) * 128)
                    if q == 0:
                        nc.scalar.activation(aggT[:H, wsl], ps[:H, :], AF.Copy)
                        if layer0:
                            nc.scalar.activation(aggT[H:H + 1, wsl], dps[:], AF.Copy)
                    else:
                        nc.vector.tensor_add(aggT[:H, wsl], aggT[:H, wsl], ps[:H, :])
                        if layer0:
                            nc.vector.tensor_add(aggT[H:H + 1, wsl],
                                                 aggT[H:H + 1, wsl], dps[:])

                win_i = 0
                for t in range(ntiles):
                    q = int(np.searchsorted(qstart, t, side="right") - 1)
                    tq = t - int(qstart[q])
                    w = int(tile_w[t])
                    Twq = int(meta["T"][w][q])
                    if t % LS_T == 0:
                        ohb = ohpool.tile([128, LS_T * 128], bf16, tag="ohb")
                        g0 = t
                        if ABL_LSCAT:
                            nc.vector.memset(ohb[:], 0.001)
                        else:
                            nc.gpsimd.local_scatter(
                                ohb[:], ew_s[:, g0:g0 + LS_T],
                                sidx_s[:, g0:g0 + LS_T],
                                channels=128, num_elems=LS_T * 128, num_idxs=LS_T)
                    if tq % CALL_T == 0:
                        nt = min(CALL_T, int(ntiles_q[q]) - tq)
                        gix = ixpool.tile([128, CALL_T * 8], i16, tag="gix")
                        nc.sync.dma_start(gix[:, :nt * 8],
                                          gidx_d[:, t * 8:(t + nt) * 8])
                        gb = gpool.tile([128, CALL_T, H2], bf16, tag="gb")
                        if ABL_GATHER:
                            nc.vector.memset(gb[:, :nt, :], 0.01)
                        else:
                            nc.gpsimd.dma_gather(
                                gb[:, :nt, :],
                                a_full[q * QROWS:(q + 1) * QROWS, :],
                                gix[:, :nt * 8], nt * 128, nt * 128, H2,
                                single_packet=False)
                    if w != cur_w or q != cur_q:
                        if cur_w >= 0:
                            flush_window(cur_q)
                        cur_w, cur_q = w, q
                        win_i = 0
                        ps = psC.tile([H, 128], f32, tag="sps")
                        if layer0:
                            dps = psD.tile([1, 128], f32, tag="dps2")
                        first_in_win = True
                        first_deg = True
                        if q == 0:
                            nc.tensor.matmul(ps[:H, :], wts[f"W3b_{l}"][:],
                                             hT[:, w * 128:(w + 1) * 128],
                                             start=True, stop=False)
                            first_in_win = False
                    oh_sl = ohb[:, (t % LS_T) * 128:(t % LS_T + 1) * 128]
                    last = win_i == Twq - 1
                    nc.tensor.matmul(ps[:H, :], gb[:, tq % CALL_T, :H], oh_sl,
                                     start=first_in_win, stop=last)
                    first_in_win = False
                    if layer0:
                        nc.tensor.matmul(dps[:], ones_col[:], oh_sl,
                                         start=first_deg, stop=last)
                        first_deg = False
                    win_i += 1
                flush_window(cur_q)

                if layer0:
                    # dgw = PE-broadcast of deg row (via partition-0 staging)
                    for ck in range(NCH):
                        sl = slice(ck * 512, (ck + 1) * 512)
                        dr = stage.tile([1, 512], f32, tag="dr")
                        nc.sync.dma_start(dr[:], aggT[H:H + 1, sl])
                        psr = psA.tile([H, 512], f32, tag="dps")
                        nc.tensor.matmul(psr[:], ones_row[:], dr[:],
                                         start=True, stop=True)
                        nc.scalar.activation(dgw[:, sl], psr[:], AF.Copy)

                # ---- combine: h = relu(aggT - t2T*dgw)
                for w in range(NW):
                    wsl = slice(w * 128, (w + 1) * 128)
                    tmp = scr.tile([H, 128], f32, tag="cmb1")
                    nc.vector.tensor_mul(tmp[:], t2T[:, wsl], dgw[:, wsl])
                    nc.vector.tensor_sub(tmp[:], aggT[:H, wsl], tmp[:])
                    if l < L - 1:
                        nc.scalar.activation(hT[:H, wsl], tmp[:], AF.Relu)
                    else:
                        hTw = scr.tile([H, 128], f32, tag="cmb2")
                        nc.scalar.activation(hTw[:], tmp[:], AF.Relu)
                        pst = psB.tile([128, H], f32, tag="tps")
                        nc.tensor.transpose(pst[:], hTw[:], ident[:H, :H])
                        nc.vector.tensor_copy(h_nm[:, w * H:(w + 1) * H], pst[:])

            # ---- pooling + MLP
            psg = psC.tile([128, H], f32, tag="sps")
            for w in range(NW):
                pw = ixpool.tile([128, 128], bf16, tag="pw")
                nc.sync.dma_start(pw[:], pool_d[:, w * 128:(w + 1) * 128])
                nc.tensor.matmul(psg[:], pw[:], h_nm[:, w * H:(w + 1) * H],
                                 start=(w == 0), stop=(w == NW - 1))
            gx = stage.tile([128, H], f32, tag="gx")
            nc.vector.tensor_copy(gx[:], psg[:])
            pst = psB.tile([128, 128], f32, tag="tps")
            nc.tensor.transpose(pst[:H, :], gx[:], ident[:])
            gxT = stage.tile([H + 1, 128], f32, tag="gxT")
            nc.vector.tensor_copy(gxT[:H, :], pst[:H, :])
            nc.vector.memset(gxT[H:H + 1, :], 1.0)
            ps1 = psB.tile([128, H], f32, tag="tps")
            nc.tensor.matmul(ps1[:], gxT[:], wts["L1b"][:], start=True, stop=True)
            r1 = stage.tile([128, H], f32, tag="r1")
            nc.scalar.activation(r1[:], ps1[:], AF.Relu)
            pst2 = psB.tile([128, 128], f32, tag="tps")
            nc.tensor.transpose(pst2[:H, :], r1[:], ident[:])
            r1T = stage.tile([H + 1, 128], f32, tag="r1T")
            nc.vector.tensor_copy(r1T[:H, :], pst2[:H, :])
            nc.vector.memset(r1T[H:H + 1, :], 1.0)
            ps2 = psB.tile([128, 4], f32, tag="tps")
            nc.tensor.matmul(ps2[:, :3], r1T[:], wts["L2b"][:], start=True, stop=True)
            outs = stage.tile([128, 4], f32, tag="outs")
            nc.vector.memset(outs[:], 0.0)
            nc.vector.tensor_copy(outs[:, :3], ps2[:, :3])
            nc.sync.dma_start(out_d[:], outs[:])

    nc.compile()
    return nc


# --------------------------------------------------------------- entry
def prepare_in_maps(inputs, cfg):
    import ml_dtypes
    per_core, meta = shard_and_pack(inputs, cfg)
    w = weights_map(inputs)
    in_maps = []
    for c in range(cfg["nc"]):
        pc = per_core[c]
        bf = ml_dtypes.bfloat16
        m = {}
        for k, v in w.items():
            m[k] = v if k in ("L1b", "L2b", "ident") else v.astype(bf)
        m["gidx"], m["sidx"] = pc["gidx"], pc["sidx"]
        m["ew"] = pc["ew"].astype(bf)
        m["xT1"] = pc["xT1"].astype(bf)
        m["pool"] = pc["pool"].astype(bf)
        in_maps.append(m)
    return in_maps, meta, per_core


def unshard_output(outs_np, out_names, out_avals, per_core, meta):
    NC = meta["cfg"]["nc"]
    i = out_names.index("out")
    arr = outs_np[i].reshape(NC, *out_avals[i].shape)
    outs = [arr[c][:per_core[c]["ngraphs"], :3] for c in range(NC)]
    return np.concatenate(outs, 0).astype(np.float32)


def run(inputs, cfg, trace=False):
    in_maps, meta, per_core = prepare_in_maps(inputs, cfg)
    nc = build_graph(meta)
    from concourse import bass_utils
    res = bass_utils.run_bass_kernel_spmd(
        nc, in_maps, core_ids=list(range(cfg["nc"])), trace=trace)
    outs = [np.asarray(res.results[c]["out"])[:per_core[c]["ngraphs"], :3]
            for c in range(cfg["nc"])]
    return np.concatenate(outs, 0).astype(np.float32), res


def kernel(**inputs):
    out, _ = run(inputs, FULL_CFG)
    return out



# revision 11
# speedup vs baseline: 1.0770x; 1.0770x over previous
"""Trainium2 Bass kernel for BA3MotifNet (4-layer LEConv GNN + mean-pool + MLP).

SPMD across 8 NeuronCores, single compiled graph; all per-core variation is
carried in the input data (index streams), never in instruction structure.

  - Nodes dst-sharded at graph boundaries (batch sorted): core c owns graphs
    [125c,125(c+1)) and their nodes, padded to NODE_PAD=12800/core.
  - Per layer: a = h@W1+b1 computed in transposed orientation, PE-transposed
    to node-major, DMA'd to DRAM, AllGather -> a_full [102400,64].
  - agg_i = sum_{e:dst=i} ew_e*a[src_e] - (h@W2)_i * degw_i.
    Gather: SWDGE dma_gather of 256B rows from a_full; int16 indices => four
    source quarters of 25600 rows; edges bucketed (quarter, window of 128
    dsts), tiled 128/tile, padded to a core-uniform tile table T[w,q].
    Scatter: PE matmul aggT[f,d] += gathered[e,f].T @ onehot[e,d] into a PSUM
    window [65,128]; onehot [128,128] bf16 built by gpsimd local_scatter.
    t3 = h@W3+b3 initializes each pass-0 psum window; deg_w rides psum row 64
    via a ones-column matmul per tile (layer 0 only).
  - h = relu(aggT + t2T*(-degw)); layer 3 also produces node-major bf16 h.
  - Mean-pool via (1/cnt)-valued one-hot matmuls; 2-layer MLP on-core.
  - Out: per-core [128,4] f32 -> host concat -> [1000,3].
"""

import os
import sys

import numpy as np

sys.path.insert(0, "/opt/trn_rl_repo")

ABL_GATHER = os.environ.get("ABL_GATHER", "0") == "1"   # replace dma_gather w/ memset
ABL_LSCAT = os.environ.get("ABL_LSCAT", "0") == "1"     # replace local_scatter w/ memset
ABL_NOAG = os.environ.get("ABL_NOAG", "0") == "1"       # replace AllGather w/ local DMA (sim only)

FULL_CFG = dict(
    n_nodes=100000, n_edges=3200000, n_graphs=1000, hid=64, n_layers=4,
    nc=8, node_pad=12800, call_tiles=36, ls_t=14,
)


# --------------------------------------------------------------- host prep
def shard_and_pack(inputs, cfg):
    NC, NP = cfg["nc"], cfg["node_pad"]
    NW, NQ = NP // 128, 4
    QROWS = NP * NC // NQ
    G = cfg["n_graphs"]
    GPC = G // NC
    assert QROWS <= 32768

    x = np.asarray(inputs["x"], np.float32)
    ei = np.asarray(inputs["edge_index"], np.int64)
    ew = np.asarray(inputs["edge_attr"], np.float32)
    batch = np.asarray(inputs["batch"], np.int64)
    N = x.shape[0]
    NF = x.shape[1]

    gs = np.searchsorted(batch, np.arange(G + 1))
    nstart = gs[np.arange(NC + 1) * GPC]
    ncnt = np.diff(nstart)
    if ncnt.max() > NP:                                    # rare: grow pad
        NP = int(-(-int(ncnt.max()) // 512) * 512)
        cfg = dict(cfg, node_pad=NP)
        NW = NP // 128
        QROWS = NP * NC // NQ
        assert QROWS <= 32768

    shard_of = np.searchsorted(nstart[1:], np.arange(N), side="right")
    src, dst = ei[0], ei[1]
    e_core = shard_of[dst]
    # quarter of a src node depends only on its shard (QROWS == 2*NP)
    e_q = shard_of[src] * NP // QROWS

    # degree-balanced window packing per core: relabel local node ids so every
    # (window, quarter) edge count is as even as possible (pulls the uniform
    # tile table T[w,q] from 9 down to 8 -> ~10% less gather/PE work).
    newloc = np.zeros(N, np.int64)
    for c in range(NC):
        n_c = int(ncnt[c])
        deg4 = np.zeros((NP, NQ), np.int64)
        selc = e_core == c
        np.add.at(deg4, (dst[selc] - nstart[c], e_q[selc]), 1)
        deg4 = deg4[:n_c]
        order = np.argsort(-deg4.sum(1), kind="stable")
        loads = np.zeros((NW, NQ), np.int64)
        fill = np.zeros(NW, np.int64)
        assign = np.zeros(n_c, np.int64)
        for n in order:
            new_loads = loads + deg4[n]
            # hard-penalize crossing the 8-tile (1024-edge) boundary; keep
            # a small safety margin, tie-break on balance
            over = np.maximum(0, new_loads - 1016).sum(1)
            cand = over * 1e6 + new_loads.max(1).astype(np.float64)
            cand[fill >= 128] = np.inf
            wsel = int(np.argmin(cand))
            assign[n] = wsel * 128 + fill[wsel]
            fill[wsel] += 1
            loads[wsel] += deg4[n]
        newloc[nstart[c]: nstart[c] + n_c] = assign

    spad = shard_of * NP + newloc
    dstloc = newloc[dst]
    e_w = dstloc >> 7

    cnt = np.zeros((NC, NW, NQ), np.int64)
    np.add.at(cnt, (e_core, e_w, e_q), 1)
    T = np.maximum(1, -(-cnt.max(axis=0) // 128))          # [NW, NQ]

    ntiles_q = T.sum(axis=0)
    ntiles = int(ntiles_q.sum())
    LS_T = cfg["ls_t"]
    ntiles_pad = -(-ntiles // LS_T) * LS_T

    tile_w = np.concatenate([np.repeat(np.arange(NW), T[:, q]) for q in range(NQ)])
    qstart_tiles = np.concatenate([[0], np.cumsum(ntiles_q)]).astype(np.int64)

    per_core = []
    for c in range(NC):
        sel = e_core == c
        s_qi = (spad[src[sel]] % QROWS).astype(np.int64)
        s_q, s_w = e_q[sel], e_w[sel]
        s_off = (dstloc[sel] & 127).astype(np.int64)
        s_ew = ew[sel]

        order = np.lexsort((s_off, s_w, s_q))
        s_qi, s_q, s_w, s_off, s_ew = (a[order] for a in (s_qi, s_q, s_w, s_off, s_ew))
        blk_sizes = (T.T.reshape(-1) * 128)
        blk_base = np.concatenate([[0], np.cumsum(blk_sizes)])[:-1].reshape(NQ, NW)
        key = s_q * NW + s_w
        grp_start = np.searchsorted(key, np.arange(NQ * NW), side="left")
        slot = blk_base[s_q, s_w] + (np.arange(key.size) - grp_start[key])

        nslots = ntiles * 128
        gidx = np.zeros(nslots, np.int16)
        ewv = np.zeros(nslots, np.float32)
        offv = np.full(nslots, -1, np.int64)
        gidx[slot] = s_qi.astype(np.int16)
        ewv[slot] = s_ew
        offv[slot] = s_off

        gw = np.tile(gidx.reshape(-1, 16).T, (8, 1))       # [128, nslots/16]

        offm = offv.reshape(ntiles, 128).T
        ewm = ewv.reshape(ntiles, 128).T
        tmod = np.arange(ntiles) % LS_T
        sidx = np.where(offm >= 0, tmod[None, :] * 128 + offm, -1).astype(np.int16)
        sidx = np.pad(sidx, ((0, 0), (0, ntiles_pad - ntiles)), constant_values=-1)
        ewm = np.pad(ewm, ((0, 0), (0, ntiles_pad - ntiles)))

        loc = newloc[nstart[c]: nstart[c + 1]]
        xT1 = np.zeros((NF + 1, NP), np.float32)
        xT1[:NF, loc] = x[nstart[c]: nstart[c + 1]].T
        xT1[NF, :] = 1.0

        nb = (batch[nstart[c]: nstart[c + 1]] - c * GPC).astype(np.int64)
        cnts = np.bincount(nb, minlength=GPC).astype(np.float32)
        pool = np.zeros((128, NP), np.float32)
        pool[loc & 127, (loc >> 7) * 128 + nb] = 1.0 / np.maximum(cnts[nb], 1.0)

        per_core.append(dict(gidx=gw, sidx=sidx, ew=ewm, xT1=xT1, pool=pool,
                             ngraphs=GPC))

    meta = dict(T=T, ntiles_q=ntiles_q.astype(int), qstart=qstart_tiles,
                tile_w=tile_w.astype(int), ntiles=ntiles, ntiles_pad=ntiles_pad,
                NW=NW, NQ=NQ, QROWS=QROWS, NF=NF, cfg=cfg)
    return per_core, meta


def weights_map(inputs):
    f32 = np.float32
    vs = np.vstack
    w = {"embWb": vs([np.asarray(inputs["emb_w"], f32),
                      np.asarray(inputs["emb_b"], f32)[None]]),
         "L1b": vs([np.asarray(inputs["lin1_w"], f32),
                    np.asarray(inputs["lin1_b"], f32)[None]]),
         "L2b": vs([np.asarray(inputs["lin2_w"], f32),
                    np.asarray(inputs["lin2_b"], f32)[None]]),
         "ident": np.eye(128, dtype=f32)}
    L = np.asarray(inputs["conv_w1"]).shape[0]
    for l in range(L):
        w[f"W1b_{l}"] = vs([np.asarray(inputs["conv_w1"][l], f32),
                            np.asarray(inputs["conv_b1"][l], f32)[None]])
        w[f"W2_{l}"] = np.asarray(inputs["conv_w2"][l], f32)
        w[f"W3b_{l}"] = vs([np.asarray(inputs["conv_w3"][l], f32),
                            np.asarray(inputs["conv_b3"][l], f32)[None]])
    return w


# --------------------------------------------------------------- builder
def build_graph(meta):
    from concourse import bacc, mybir, tile

    cfg = meta["cfg"]
    NC, H, L = cfg["nc"], cfg["hid"], cfg["n_layers"]
    NP, NW, NQ, QROWS = cfg["node_pad"], meta["NW"], meta["NQ"], meta["QROWS"]
    NF = meta["NF"]
    H2 = 2 * H                                  # padded bf16 a-row (256B)
    ntiles, ntiles_pad = meta["ntiles"], meta["ntiles_pad"]
    tile_w, qstart, ntiles_q = meta["tile_w"], meta["qstart"], meta["ntiles_q"]
    CALL_T, LS_T = cfg["call_tiles"], cfg["ls_t"]
    f32, bf16, i16 = mybir.dt.float32, mybir.dt.bfloat16, mybir.dt.int16
    AF = mybir.ActivationFunctionType
    NCH = NP // 512

    nc = bacc.Bacc(num_devices=NC, num_swdge_queues=4)

    gidx_d = nc.declare_dram_parameter("gidx", [128, ntiles * 8], i16, False)
    sidx_d = nc.declare_dram_parameter("sidx", [128, ntiles_pad], i16, False)
    ew_d = nc.declare_dram_parameter("ew", [128, ntiles_pad], bf16, False)
    xT1_d = nc.declare_dram_parameter("xT1", [NF + 1, NP], bf16, False)
    pool_d = nc.declare_dram_parameter("pool", [128, NP], bf16, False)
    wnames = (["embWb", "L1b", "L2b", "ident"]
              + [f"{p}_{l}" for l in range(L) for p in ("W1b", "W2", "W3b")])
    wshape = {"embWb": [NF + 1, H], "L1b": [H + 1, H], "L2b": [H + 1, 3],
              "ident": [128, 128]}
    wdt = {"embWb": bf16, "L1b": f32, "L2b": f32, "ident": f32}
    for l in range(L):
        wshape[f"W1b_{l}"] = [H + 1, H]
        wshape[f"W2_{l}"] = [H, H]
        wshape[f"W3b_{l}"] = [H + 1, H]
        wdt[f"W1b_{l}"] = wdt[f"W2_{l}"] = wdt[f"W3b_{l}"] = bf16
    wd = {k: nc.declare_dram_parameter(k, wshape[k], wdt[k], False)
          for k in wnames}
    out_d = nc.declare_dram_parameter("out", [128, 4], f32, True)

    with tile.TileContext(nc) as tc:
        with (
            tc.tile_pool(name="res", bufs=1) as res,
            tc.tile_pool(name="dram", bufs=1, space="DRAM") as dram,
            tc.tile_pool(name="stage", bufs=2) as stage,
            tc.tile_pool(name="gbuf", bufs=3) as gpool,
            tc.tile_pool(name="ohbuf", bufs=2) as ohpool,
            tc.tile_pool(name="ixbuf", bufs=3) as ixpool,
            tc.tile_pool(name="scr", bufs=3) as scr,
            tc.tile_pool(name="psA", bufs=2, space="PSUM") as psA,
            tc.tile_pool(name="psB", bufs=2, space="PSUM") as psB,
            tc.tile_pool(name="psC", bufs=2, space="PSUM") as psC,
            tc.tile_pool(name="psD", bufs=2, space="PSUM") as psD,
        ):
            a_loc = dram.tile([NP, H2], bf16)
            a_full = dram.tile([NP * NC, H2], bf16)

            hT = res.tile([H + 1, NP], bf16, tag="hT")
            t2T = res.tile([H, NP], bf16, tag="t2T")
            aggT = res.tile([H + 1, NP], f32, tag="aggT")
            dgw = res.tile([H, NP], bf16, tag="dgw")
            sidx_s = res.tile([128, ntiles_pad], i16, tag="sidx")
            ew_s = res.tile([128, ntiles_pad], bf16, tag="ew")
            h_nm = res.tile([128, NW * H], bf16, tag="h_nm")
            ones_col = res.tile([128, 1], bf16, tag="ones")
            ones_row = res.tile([1, H], f32, tag="ones_row")
            wts = {k: res.tile(wshape[k], wdt[k], tag=k, name=k) for k in wnames}
            ident = wts["ident"]

            nc.vector.memset(ones_col[:], 1.0)
            nc.vector.memset(ones_row[:], 1.0)
            for k in wnames:
                nc.sync.dma_start(wts[k][:], wd[k][:])
            nc.sync.dma_start(sidx_s[:], sidx_d[:])
            nc.sync.dma_start(ew_s[:], ew_d[:])

            # h0T = embWb.T @ xT1 (streamed)
            for ck in range(NCH):
                sl = slice(ck * 512, (ck + 1) * 512)
                xc = stage.tile([NF + 1, 512], bf16, tag="xc")
                nc.sync.dma_start(xc[:], xT1_d[:, sl])
                ps = psA.tile([H, 512], f32, tag="dps")
                nc.tensor.matmul(ps[:], wts["embWb"][:], xc[:],
                                 start=True, stop=True)
                nc.scalar.activation(hT[:H, sl], ps[:], AF.Copy)
            nc.vector.memset(hT[H:H + 1, :], 1.0)

            for l in range(L):
                layer0 = l == 0
                # ---- dense: a -> a_loc (node-major bf16, 128-padded), t2T
                for ck in range(NCH):
                    sl = slice(ck * 512, (ck + 1) * 512)
                    ps = psA.tile([H, 512], f32, tag="dps")
                    nc.tensor.matmul(ps[:], wts[f"W1b_{l}"][:], hT[:, sl],
                                     start=True, stop=True)
                    aTc = stage.tile([H, 512], f32, tag="aTc")
                    nc.scalar.activation(aTc[:], ps[:], AF.Copy)
                    ast = stage.tile([128, 4, H2], bf16, tag="ast")
                    nc.vector.memset(ast[:, :, H:], 0.0)
                    for j in range(4):
                        pst = psB.tile([128, H], f32, tag="tps")
                        nc.tensor.transpose(pst[:], aTc[:, j * 128:(j + 1) * 128],
                                            ident[:H, :H])
                        nc.vector.tensor_copy(ast[:, j, :H], pst[:])
                    nc.sync.dma_start(
                        a_loc[sl, :].rearrange("(j p) f -> p j f", p=128), ast[:])
                    ps2 = psA.tile([H, 512], f32, tag="dps")
                    nc.tensor.matmul(ps2[:], wts[f"W2_{l}"][:], hT[:H, sl],
                                     start=True, stop=True)
                    nc.scalar.activation(t2T[:, sl], ps2[:], AF.Copy)

                if ABL_NOAG:
                    nc.sync.dma_start(a_full[:NP, :], a_loc[:])
                else:
                    nc.gpsimd.collective_compute(
                        "AllGather", mybir.AluOpType.bypass,
                        replica_groups=[list(range(NC))],
                        ins=[a_loc[:].opt()], outs=[a_full[:].opt()])

                # ---- gather + scatter: walk global tile stream
                ohb = None
                gb = None
                cur_w, cur_q = -1, -1
                ps = None
                dps = None
                first_in_win = False

                def flush_window(q):
                    wsl = slice(cur_w * 128, (cur_w + 1) * 128)
                    if q == 0:
                        nc.scalar.activation(aggT[:H, wsl], ps[:H, :], AF.Copy)
                        if layer0:
                            nc.scalar.activation(aggT[H:H + 1, wsl], dps[:], AF.Copy)
                    else:
                        nc.vector.tensor_add(aggT[:H, wsl], aggT[:H, wsl], ps[:H, :])
                        if layer0:
                            nc.vector.tensor_add(aggT[H:H + 1, wsl],
                                                 aggT[H:H + 1, wsl], dps[:])

                win_i = 0
                call_i = 0
                for t in range(ntiles):
                    q = int(np.searchsorted(qstart, t, side="right") - 1)
                    tq = t - int(qstart[q])
                    w = int(tile_w[t])
                    Twq = int(meta["T"][w][q])
                    if t % LS_T == 0:
                        ohb = ohpool.tile([128, LS_T * 128], bf16, tag="ohb")
                        g0 = t
                        if ABL_LSCAT:
                            nc.vector.memset(ohb[:], 0.001)
                        else:
                            nc.gpsimd.local_scatter(
                                ohb[:], ew_s[:, g0:g0 + LS_T],
                                sidx_s[:, g0:g0 + LS_T],
                                channels=128, num_elems=LS_T * 128, num_idxs=LS_T)
                    if tq % CALL_T == 0:
                        nt = min(CALL_T, int(ntiles_q[q]) - tq)
                        gix = ixpool.tile([128, CALL_T * 8], i16, tag="gix")
                        nc.sync.dma_start(gix[:, :nt * 8],
                                          gidx_d[:, t * 8:(t + nt) * 8])
                        gb = gpool.tile([128, CALL_T, H2], bf16, tag="gb")
                        if ABL_GATHER:
                            nc.vector.memset(gb[:, :nt, :], 0.01)
                        else:
                            nc.gpsimd.dma_gather(
                                gb[:, :nt, :],
                                a_full[q * QROWS:(q + 1) * QROWS, :],
                                gix[:, :nt * 8], nt * 128, nt * 128, H2,
                                single_packet=False,
                                queue_num=call_i % 4)
                        call_i += 1
                    if w != cur_w or q != cur_q:
                        if cur_w >= 0:
                            flush_window(cur_q)
                        cur_w, cur_q = w, q
                        win_i = 0
                        ps = psC.tile([H, 128], f32, tag="sps")
                        if layer0:
                            dps = psD.tile([1, 128], f32, tag="dps2")
                        first_in_win = True
                        first_deg = True
                        if q == 0:
                            nc.tensor.matmul(ps[:H, :], wts[f"W3b_{l}"][:],
                                             hT[:, w * 128:(w + 1) * 128],
                                             start=True, stop=False)
                            first_in_win = False
                    oh_sl = ohb[:, (t % LS_T) * 128:(t % LS_T + 1) * 128]
                    last = win_i == Twq - 1
                    nc.tensor.matmul(ps[:H, :], gb[:, tq % CALL_T, :H], oh_sl,
                                     start=first_in_win, stop=last)
                    first_in_win = False
                    if layer0:
                        nc.tensor.matmul(dps[:], ones_col[:], oh_sl,
                                         start=first_deg, stop=last)
                        first_deg = False
                    win_i += 1
                flush_window(cur_q)

                if layer0:
                    # dgw = PE-broadcast of deg row (via partition-0 staging)
                    for ck in range(NCH):
                        sl = slice(ck * 512, (ck + 1) * 512)
                        dr = stage.tile([1, 512], f32, tag="dr")
                        nc.sync.dma_start(dr[:], aggT[H:H + 1, sl])
                        psr = psA.tile([H, 512], f32, tag="dps")
                        nc.tensor.matmul(psr[:], ones_row[:], dr[:],
                                         start=True, stop=True)
                        nc.scalar.activation(dgw[:, sl], psr[:], AF.Copy)

                # ---- combine: h = relu(aggT - t2T*dgw)
                for w in range(NW):
                    wsl = slice(w * 128, (w + 1) * 128)
                    tmp = scr.tile([H, 128], f32, tag="cmb1")
                    nc.vector.tensor_mul(tmp[:], t2T[:, wsl], dgw[:, wsl])
                    nc.vector.tensor_sub(tmp[:], aggT[:H, wsl], tmp[:])
                    if l < L - 1:
                        nc.scalar.activation(hT[:H, wsl], tmp[:], AF.Relu)
                    else:
                        hTw = scr.tile([H, 128], f32, tag="cmb2")
                        nc.scalar.activation(hTw[:], tmp[:], AF.Relu)
                        pst = psB.tile([128, H], f32, tag="tps")
                        nc.tensor.transpose(pst[:], hTw[:], ident[:H, :H])
                        nc.vector.tensor_copy(h_nm[:, w * H:(w + 1) * H], pst[:])

            # ---- pooling + MLP
            psg = psC.tile([128, H], f32, tag="sps")
            for w in range(NW):
                pw = ixpool.tile([128, 128], bf16, tag="pw")
                nc.sync.dma_start(pw[:], pool_d[:, w * 128:(w + 1) * 128])
                nc.tensor.matmul(psg[:], pw[:], h_nm[:, w * H:(w + 1) * H],
                                 start=(w == 0), stop=(w == NW - 1))
            gx = stage.tile([128, H], f32, tag="gx")
            nc.vector.tensor_copy(gx[:], psg[:])
            pst = psB.tile([128, 128], f32, tag="tps")
            nc.tensor.transpose(pst[:H, :], gx[:], ident[:])
            gxT = stage.tile([H + 1, 128], f32, tag="gxT")
            nc.vector.tensor_copy(gxT[:H, :], pst[:H, :])
            nc.vector.memset(gxT[H:H + 1, :], 1.0)
            ps1 = psB.tile([128, H], f32, tag="tps")
            nc.tensor.matmul(ps1[:], gxT[:], wts["L1b"][:], start=True, stop=True)
            r1 = stage.tile([128, H], f32, tag="r1")
            nc.scalar.activation(r1[:], ps1[:], AF.Relu)
            pst2 = psB.tile([128, 128], f32, tag="tps")
            nc.tensor.transpose(pst2[:H, :], r1[:], ident[:])
            r1T = stage.tile([H + 1, 128], f32, tag="r1T")
            nc.vector.tensor_copy(r1T[:H, :], pst2[:H, :])
            nc.vector.memset(r1T[H:H + 1, :], 1.0)
            ps2 = psB.tile([128, 4], f32, tag="tps")
            nc.tensor.matmul(ps2[:, :3], r1T[:], wts["L2b"][:], start=True, stop=True)
            outs = stage.tile([128, 4], f32, tag="outs")
            nc.vector.memset(outs[:], 0.0)
            nc.vector.tensor_copy(outs[:, :3], ps2[:, :3])
            nc.sync.dma_start(out_d[:], outs[:])

    nc.compile()
    return nc


# --------------------------------------------------------------- entry
def prepare_in_maps(inputs, cfg):
    import ml_dtypes
    per_core, meta = shard_and_pack(inputs, cfg)
    w = weights_map(inputs)
    in_maps = []
    for c in range(cfg["nc"]):
        pc = per_core[c]
        bf = ml_dtypes.bfloat16
        m = {}
        for k, v in w.items():
            m[k] = v if k in ("L1b", "L2b", "ident") else v.astype(bf)
        m["gidx"], m["sidx"] = pc["gidx"], pc["sidx"]
        m["ew"] = pc["ew"].astype(bf)
        m["xT1"] = pc["xT1"].astype(bf)
        m["pool"] = pc["pool"].astype(bf)
        in_maps.append(m)
    return in_maps, meta, per_core


def unshard_output(outs_np, out_names, out_avals, per_core, meta):
    NC = meta["cfg"]["nc"]
    i = out_names.index("out")
    arr = outs_np[i].reshape(NC, *out_avals[i].shape)
    outs = [arr[c][:per_core[c]["ngraphs"], :3] for c in range(NC)]
    return np.concatenate(outs, 0).astype(np.float32)


def run(inputs, cfg, trace=False):
    in_maps, meta, per_core = prepare_in_maps(inputs, cfg)
    nc = build_graph(meta)
    from concourse import bass_utils
    res = bass_utils.run_bass_kernel_spmd(
        nc, in_maps, core_ids=list(range(cfg["nc"])), trace=trace)
    outs = [np.asarray(res.results[c]["out"])[:per_core[c]["ngraphs"], :3]
            for c in range(cfg["nc"])]
    return np.concatenate(outs, 0).astype(np.float32), res


def kernel(**inputs):
    out, _ = run(inputs, FULL_CFG)
    return out



# revision 20
# speedup vs baseline: 1.2468x; 1.1577x over previous
"""Trainium2 Bass kernel for BA3MotifNet (4-layer LEConv GNN + mean-pool + MLP).

SPMD across 8 NeuronCores, single compiled graph; all per-core variation is
carried in the input data (index streams), never in instruction structure.

  - Nodes dst-sharded at graph boundaries (batch sorted): core c owns graphs
    [125c,125(c+1)) and their nodes, padded to NODE_PAD=12800/core.
  - Per layer: a = h@W1+b1 computed in transposed orientation, PE-transposed
    to node-major, DMA'd to DRAM, AllGather -> a_full [102400,64].
  - agg_i = sum_{e:dst=i} ew_e*a[src_e] - (h@W2)_i * degw_i.
    Gather: SWDGE dma_gather of 256B rows from a_full; int16 indices => four
    source quarters of 25600 rows; edges bucketed (quarter, window of 128
    dsts), tiled 128/tile, padded to a core-uniform tile table T[w,q].
    Scatter: PE matmul aggT[f,d] += gathered[e,f].T @ onehot[e,d] into a PSUM
    window [65,128]; onehot [128,128] bf16 built by gpsimd local_scatter.
    t3 = h@W3+b3 initializes each pass-0 psum window; deg_w rides psum row 64
    via a ones-column matmul per tile (layer 0 only).
  - h = relu(aggT + t2T*(-degw)); layer 3 also produces node-major bf16 h.
  - Mean-pool via (1/cnt)-valued one-hot matmuls; 2-layer MLP on-core.
  - Out: per-core [128,4] f32 -> host concat -> [1000,3].
"""

import os
import sys

import numpy as np

sys.path.insert(0, "/opt/trn_rl_repo")

ABL_GATHER = os.environ.get("ABL_GATHER", "0") == "1"   # replace dma_gather w/ memset
ABL_LSCAT = os.environ.get("ABL_LSCAT", "0") == "1"     # replace local_scatter w/ memset
ABL_NOAG = os.environ.get("ABL_NOAG", "0") == "1"       # replace AllGather w/ local DMA (sim only)

FULL_CFG = dict(
    n_nodes=100000, n_edges=3200000, n_graphs=1000, hid=64, n_layers=4,
    nc=8, node_pad=12800, call_tiles=32, ls_t=14,
)


# --------------------------------------------------------------- host prep
def shard_and_pack(inputs, cfg):
    NC, NP = cfg["nc"], cfg["node_pad"]
    NW, NQ = NP // 128, 4
    QROWS = NP * NC // NQ
    G = cfg["n_graphs"]
    GPC = G // NC
    assert QROWS <= 32768

    x = np.asarray(inputs["x"], np.float32)
    ei = np.asarray(inputs["edge_index"], np.int64)
    ew = np.asarray(inputs["edge_attr"], np.float32)
    batch = np.asarray(inputs["batch"], np.int64)
    N = x.shape[0]
    NF = x.shape[1]

    gs = np.searchsorted(batch, np.arange(G + 1))
    nstart = gs[np.arange(NC + 1) * GPC]
    ncnt = np.diff(nstart)
    if ncnt.max() > NP:                                    # rare: grow pad
        NP = int(-(-int(ncnt.max()) // 512) * 512)
        cfg = dict(cfg, node_pad=NP)
        NW = NP // 128
        QROWS = NP * NC // NQ
        assert QROWS <= 32768

    shard_of = np.searchsorted(nstart[1:], np.arange(N), side="right")
    src, dst = ei[0], ei[1]
    e_core = shard_of[dst]
    # quarter of a src node depends only on its shard (QROWS == 2*NP)
    e_q = shard_of[src] * NP // QROWS

    # degree-balanced window packing per core: relabel local node ids so every
    # (window, quarter) edge count is as even as possible (pulls the uniform
    # tile table T[w,q] from 9 down to 8 -> ~10% less gather/PE work).
    newloc = np.zeros(N, np.int64)
    for c in range(NC):
        n_c = int(ncnt[c])
        deg4 = np.zeros((NP, NQ), np.int64)
        selc = e_core == c
        np.add.at(deg4, (dst[selc] - nstart[c], e_q[selc]), 1)
        deg4 = deg4[:n_c]
        order = np.argsort(-deg4.sum(1), kind="stable")
        loads = np.zeros((NW, NQ), np.int64)
        fill = np.zeros(NW, np.int64)
        assign = np.zeros(n_c, np.int64)
        for n in order:
            new_loads = loads + deg4[n]
            # hard-penalize crossing the 8-tile (1024-edge) boundary; keep
            # a small safety margin, tie-break on balance
            over = np.maximum(0, new_loads - 1016).sum(1)
            cand = over * 1e6 + new_loads.max(1).astype(np.float64)
            cand[fill >= 128] = np.inf
            wsel = int(np.argmin(cand))
            assign[n] = wsel * 128 + fill[wsel]
            fill[wsel] += 1
            loads[wsel] += deg4[n]
        newloc[nstart[c]: nstart[c] + n_c] = assign

    spad = shard_of * NP + newloc
    dstloc = newloc[dst]
    e_w = dstloc >> 7

    cnt = np.zeros((NC, NW, NQ), np.int64)
    np.add.at(cnt, (e_core, e_w, e_q), 1)
    T = np.maximum(1, -(-cnt.max(axis=0) // 128))          # [NW, NQ]

    ntiles_q = T.sum(axis=0)
    ntiles = int(ntiles_q.sum())
    LS_T = cfg["ls_t"]
    ntiles_pad = -(-ntiles // LS_T) * LS_T

    tile_w = np.concatenate([np.repeat(np.arange(NW), T[:, q]) for q in range(NQ)])
    qstart_tiles = np.concatenate([[0], np.cumsum(ntiles_q)]).astype(np.int64)

    per_core = []
    for c in range(NC):
        sel = e_core == c
        s_qi = (spad[src[sel]] % QROWS).astype(np.int64)
        s_q, s_w = e_q[sel], e_w[sel]
        s_off = (dstloc[sel] & 127).astype(np.int64)
        s_ew = ew[sel]

        order = np.lexsort((s_off, s_w, s_q))
        s_qi, s_q, s_w, s_off, s_ew = (a[order] for a in (s_qi, s_q, s_w, s_off, s_ew))
        blk_sizes = (T.T.reshape(-1) * 128)
        blk_base = np.concatenate([[0], np.cumsum(blk_sizes)])[:-1].reshape(NQ, NW)
        key = s_q * NW + s_w
        grp_start = np.searchsorted(key, np.arange(NQ * NW), side="left")
        slot = blk_base[s_q, s_w] + (np.arange(key.size) - grp_start[key])

        nslots = ntiles * 128
        gidx = np.zeros(nslots, np.int16)
        ewv = np.zeros(nslots, np.float32)
        offv = np.full(nslots, -1, np.int64)
        gidx[slot] = s_qi.astype(np.int16)
        ewv[slot] = s_ew
        offv[slot] = s_off

        gw = np.tile(gidx.reshape(-1, 16).T, (8, 1))       # [128, nslots/16]

        offm = offv.reshape(ntiles, 128).T
        ewm = ewv.reshape(ntiles, 128).T
        tmod = np.arange(ntiles) % LS_T
        sidx = np.where(offm >= 0, tmod[None, :] * 128 + offm, -1).astype(np.int16)
        sidx = np.pad(sidx, ((0, 0), (0, ntiles_pad - ntiles)), constant_values=-1)
        ewm = np.pad(ewm, ((0, 0), (0, ntiles_pad - ntiles)))

        loc = newloc[nstart[c]: nstart[c + 1]]
        xT1 = np.zeros((NF + 1, NP), np.float32)
        xT1[:NF, loc] = x[nstart[c]: nstart[c + 1]].T
        xT1[NF, :] = 1.0

        nb = (batch[nstart[c]: nstart[c + 1]] - c * GPC).astype(np.int64)
        cnts = np.bincount(nb, minlength=GPC).astype(np.float32)
        pool = np.zeros((128, NP), np.float32)
        pool[loc & 127, (loc >> 7) * 128 + nb] = 1.0 / np.maximum(cnts[nb], 1.0)

        iotac = np.tile(np.arange(LS_T * 128, dtype=np.int16), (128, 1))
        per_core.append(dict(gidx=gw, sidx=sidx, ew=ewm, xT1=xT1, pool=pool,
                             iotac=iotac, ngraphs=GPC))

    meta = dict(T=T, ntiles_q=ntiles_q.astype(int), qstart=qstart_tiles,
                tile_w=tile_w.astype(int), ntiles=ntiles, ntiles_pad=ntiles_pad,
                NW=NW, NQ=NQ, QROWS=QROWS, NF=NF, cfg=cfg)
    return per_core, meta


def weights_map(inputs):
    f32 = np.float32
    vs = np.vstack
    w = {"embWb": vs([np.asarray(inputs["emb_w"], f32),
                      np.asarray(inputs["emb_b"], f32)[None]]),
         "L1b": vs([np.asarray(inputs["lin1_w"], f32),
                    np.asarray(inputs["lin1_b"], f32)[None]]),
         "L2b": vs([np.asarray(inputs["lin2_w"], f32),
                    np.asarray(inputs["lin2_b"], f32)[None]]),
         "ident": np.eye(128, dtype=f32)}
    L = np.asarray(inputs["conv_w1"]).shape[0]
    for l in range(L):
        w[f"W1b_{l}"] = vs([np.asarray(inputs["conv_w1"][l], f32),
                            np.asarray(inputs["conv_b1"][l], f32)[None]])
        w[f"W2_{l}"] = np.asarray(inputs["conv_w2"][l], f32)
        w[f"W3b_{l}"] = vs([np.asarray(inputs["conv_w3"][l], f32),
                            np.asarray(inputs["conv_b3"][l], f32)[None]])
    return w


# --------------------------------------------------------------- builder
def build_graph(meta):
    from concourse import bacc, mybir, tile

    cfg = meta["cfg"]
    NC, H, L = cfg["nc"], cfg["hid"], cfg["n_layers"]
    NP, NW, NQ, QROWS = cfg["node_pad"], meta["NW"], meta["NQ"], meta["QROWS"]
    NF = meta["NF"]
    H2 = 2 * H                                  # padded bf16 a-row (256B)
    ntiles, ntiles_pad = meta["ntiles"], meta["ntiles_pad"]
    tile_w, qstart, ntiles_q = meta["tile_w"], meta["qstart"], meta["ntiles_q"]
    CALL_T, LS_T = cfg["call_tiles"], cfg["ls_t"]
    f32, bf16, i16 = mybir.dt.float32, mybir.dt.bfloat16, mybir.dt.int16
    AF = mybir.ActivationFunctionType
    NCH = NP // 512

    nc = bacc.Bacc(num_devices=NC, num_swdge_queues=4)

    gidx_d = nc.declare_dram_parameter("gidx", [128, ntiles * 8], i16, False)
    sidx_d = nc.declare_dram_parameter("sidx", [128, ntiles_pad], i16, False)
    iotac_d = nc.declare_dram_parameter("iotac", [128, LS_T * 128], i16, False)
    ew_d = nc.declare_dram_parameter("ew", [128, ntiles_pad], bf16, False)
    xT1_d = nc.declare_dram_parameter("xT1", [NF + 1, NP], bf16, False)
    pool_d = nc.declare_dram_parameter("pool", [128, NP], bf16, False)
    wnames = (["embWb", "L1b", "L2b", "ident"]
              + [f"{p}_{l}" for l in range(L) for p in ("W1b", "W2", "W3b")])
    wshape = {"embWb": [NF + 1, H], "L1b": [H + 1, H], "L2b": [H + 1, 3],
              "ident": [128, 128]}
    wdt = {"embWb": bf16, "L1b": f32, "L2b": f32, "ident": f32}
    for l in range(L):
        wshape[f"W1b_{l}"] = [H + 1, H]
        wshape[f"W2_{l}"] = [H, H]
        wshape[f"W3b_{l}"] = [H + 1, H]
        wdt[f"W1b_{l}"] = wdt[f"W2_{l}"] = wdt[f"W3b_{l}"] = bf16
    wd = {k: nc.declare_dram_parameter(k, wshape[k], wdt[k], False)
          for k in wnames}
    out_d = nc.declare_dram_parameter("out", [128, 4], f32, True)

    with tile.TileContext(nc) as tc:
        with (
            tc.tile_pool(name="res", bufs=1) as res,
            tc.tile_pool(name="dram", bufs=1, space="DRAM") as dram,
            tc.tile_pool(name="stage", bufs=2) as stage,
            tc.tile_pool(name="gbuf", bufs=3) as gpool,
            tc.tile_pool(name="ohbuf", bufs=2) as ohpool,
            tc.tile_pool(name="ixbuf", bufs=2) as ixpool,
            tc.tile_pool(name="scr", bufs=3) as scr,
            tc.tile_pool(name="psA", bufs=2, space="PSUM") as psA,
            tc.tile_pool(name="psB", bufs=2, space="PSUM") as psB,
            tc.tile_pool(name="psC", bufs=2, space="PSUM") as psC,
            tc.tile_pool(name="psD", bufs=2, space="PSUM") as psD,
        ):
            a_loc = dram.tile([NP, H2], bf16)
            a_full = dram.tile([NP * NC, H2], bf16)

            hT = res.tile([H + 1, NP], bf16, tag="hT")
            t2T = res.tile([H, NP], bf16, tag="t2T")
            aggT = res.tile([H + 1, NP], f32, tag="aggT")
            dgw = res.tile([H, NP], bf16, tag="dgw")
            sidx_s = res.tile([128, ntiles_pad], i16, tag="sidx")
            ew_s = res.tile([128, ntiles_pad], bf16, tag="ew")
            iotaC = res.tile([128, LS_T, 128], i16, tag="iotac")
            h_nm = res.tile([128, NW * H], bf16, tag="h_nm")
            ones_col = res.tile([128, 1], bf16, tag="ones")
            ones_row = res.tile([1, H], f32, tag="ones_row")
            wts = {k: res.tile(wshape[k], wdt[k], tag=k, name=k) for k in wnames}
            ident = wts["ident"]

            nc.vector.memset(ones_col[:], 1.0)
            nc.vector.memset(ones_row[:], 1.0)
            for k in wnames:
                nc.sync.dma_start(wts[k][:], wd[k][:])
            nc.sync.dma_start(sidx_s[:], sidx_d[:])
            nc.sync.dma_start(ew_s[:], ew_d[:])
            nc.sync.dma_start(
                iotaC[:].rearrange("p t d -> p (t d)"), iotac_d[:])

            # h0T = embWb.T @ xT1 (streamed)
            for ck in range(NCH):
                sl = slice(ck * 512, (ck + 1) * 512)
                xc = stage.tile([NF + 1, 512], bf16, tag="xc")
                nc.sync.dma_start(xc[:], xT1_d[:, sl])
                ps = psA.tile([H, 512], f32, tag="dps")
                nc.tensor.matmul(ps[:], wts["embWb"][:], xc[:],
                                 start=True, stop=True)
                nc.scalar.activation(hT[:H, sl], ps[:], AF.Copy)
            nc.vector.memset(hT[H:H + 1, :], 1.0)

            for l in range(L):
                layer0 = l == 0
                # ---- dense: a -> a_loc (node-major bf16, 128-padded), t2T
                for ck in range(NCH):
                    sl = slice(ck * 512, (ck + 1) * 512)
                    ps = psA.tile([H, 512], f32, tag="dps")
                    nc.tensor.matmul(ps[:], wts[f"W1b_{l}"][:], hT[:, sl],
                                     start=True, stop=True)
                    aTc = stage.tile([H, 512], f32, tag="aTc")
                    nc.scalar.activation(aTc[:], ps[:], AF.Copy)
                    ast = stage.tile([128, 4, H2], bf16, tag="ast")
                    nc.vector.memset(ast[:, :, H:], 0.0)
                    for j in range(4):
                        pst = psB.tile([128, H], f32, tag="tps")
                        nc.tensor.transpose(pst[:], aTc[:, j * 128:(j + 1) * 128],
                                            ident[:H, :H])
                        nc.vector.tensor_copy(ast[:, j, :H], pst[:])
                    nc.sync.dma_start(
                        a_loc[sl, :].rearrange("(j p) f -> p j f", p=128), ast[:])
                    ps2 = psA.tile([H, 512], f32, tag="dps")
                    nc.tensor.matmul(ps2[:], wts[f"W2_{l}"][:], hT[:H, sl],
                                     start=True, stop=True)
                    nc.scalar.activation(t2T[:, sl], ps2[:], AF.Copy)

                if ABL_NOAG:
                    nc.sync.dma_start(a_full[:NP, :], a_loc[:])
                else:
                    nc.gpsimd.collective_compute(
                        "AllGather", mybir.AluOpType.bypass,
                        replica_groups=[list(range(NC))],
                        ins=[a_loc[:].opt()], outs=[a_full[:].opt()])

                # ---- gather + scatter: walk global tile stream
                ohb = None
                gb = None
                cur_w, cur_q = -1, -1
                ps = None
                dps = None
                first_in_win = False

                def flush_window(q):
                    wsl = slice(cur_w * 128, (cur_w + 1) * 128)
                    if q == 0:
                        nc.scalar.activation(aggT[:H, wsl], ps[:H, :], AF.Copy)
                        if layer0:
                            nc.scalar.activation(aggT[H:H + 1, wsl], dps[:], AF.Copy)
                    else:
                        nc.vector.tensor_add(aggT[:H, wsl], aggT[:H, wsl], ps[:H, :])
                        if layer0:
                            nc.vector.tensor_add(aggT[H:H + 1, wsl],
                                                 aggT[H:H + 1, wsl], dps[:])

                win_i = 0
                call_i = 0
                for t in range(ntiles):
                    q = int(np.searchsorted(qstart, t, side="right") - 1)
                    tq = t - int(qstart[q])
                    w = int(tile_w[t])
                    Twq = int(meta["T"][w][q])
                    if t % LS_T == 0:
                        ohb = ohpool.tile([128, LS_T, 128], bf16, tag="ohb")
                        g0 = t
                        if ABL_LSCAT:
                            nc.vector.memset(ohb[:], 0.001)
                        else:
                            nc.vector.tensor_tensor(
                                ohb[:], iotaC[:],
                                sidx_s[:, g0:g0 + LS_T][:, :, None]
                                .broadcast_to([128, LS_T, 128]),
                                op=mybir.AluOpType.is_equal)
                            nc.vector.tensor_mul(
                                ohb[:], ohb[:],
                                ew_s[:, g0:g0 + LS_T][:, :, None]
                                .broadcast_to([128, LS_T, 128]))
                    if tq % CALL_T == 0:
                        nt = min(CALL_T, int(ntiles_q[q]) - tq)
                        gix = ixpool.tile([128, CALL_T * 8], i16, tag="gix")
                        nc.sync.dma_start(gix[:, :nt * 8],
                                          gidx_d[:, t * 8:(t + nt) * 8])
                        gb = gpool.tile([128, CALL_T, H2], bf16, tag="gb")
                        if ABL_GATHER:
                            nc.vector.memset(gb[:, :nt, :], 0.01)
                        else:
                            nc.gpsimd.dma_gather(
                                gb[:, :nt, :],
                                a_full[q * QROWS:(q + 1) * QROWS, :],
                                gix[:, :nt * 8], nt * 128, nt * 128, H2,
                                single_packet=False,
                                queue_num=call_i % 4)
                        call_i += 1
                    if w != cur_w or q != cur_q:
                        if cur_w >= 0:
                            flush_window(cur_q)
                        cur_w, cur_q = w, q
                        win_i = 0
                        ps = psC.tile([H, 128], f32, tag="sps")
                        if layer0:
                            dps = psD.tile([1, 128], f32, tag="dps2")
                        first_in_win = True
                        first_deg = True
                        if q == 0:
                            nc.tensor.matmul(ps[:H, :], wts[f"W3b_{l}"][:],
                                             hT[:, w * 128:(w + 1) * 128],
                                             start=True, stop=False)
                            first_in_win = False
                    oh_sl = ohb[:, t % LS_T, :]
                    last = win_i == Twq - 1
                    nc.tensor.matmul(ps[:H, :], gb[:, tq % CALL_T, :H], oh_sl,
                                     start=first_in_win, stop=last)
                    first_in_win = False
                    if layer0:
                        nc.tensor.matmul(dps[:], ones_col[:], oh_sl,
                                         start=first_deg, stop=last)
                        first_deg = False
                    win_i += 1
                flush_window(cur_q)

                if layer0:
                    # dgw = PE-broadcast of deg row (via partition-0 staging)
                    for ck in range(NCH):
                        sl = slice(ck * 512, (ck + 1) * 512)
                        dr = stage.tile([1, 512], f32, tag="dr")
                        nc.sync.dma_start(dr[:], aggT[H:H + 1, sl])
                        psr = psA.tile([H, 512], f32, tag="dps")
                        nc.tensor.matmul(psr[:], ones_row[:], dr[:],
                                         start=True, stop=True)
                        nc.scalar.activation(dgw[:, sl], psr[:], AF.Copy)

                # ---- combine: h = relu(aggT - t2T*dgw)
                for w in range(NW):
                    wsl = slice(w * 128, (w + 1) * 128)
                    tmp = scr.tile([H, 128], f32, tag="cmb1")
                    nc.vector.tensor_mul(tmp[:], t2T[:, wsl], dgw[:, wsl])
                    nc.vector.tensor_sub(tmp[:], aggT[:H, wsl], tmp[:])
                    if l < L - 1:
                        nc.scalar.activation(hT[:H, wsl], tmp[:], AF.Relu)
                    else:
                        hTw = scr.tile([H, 128], f32, tag="cmb2")
                        nc.scalar.activation(hTw[:], tmp[:], AF.Relu)
                        pst = psB.tile([128, H], f32, tag="tps")
                        nc.tensor.transpose(pst[:], hTw[:], ident[:H, :H])
                        nc.vector.tensor_copy(h_nm[:, w * H:(w + 1) * H], pst[:])

            # ---- pooling + MLP
            psg = psC.tile([128, H], f32, tag="sps")
            for w in range(NW):
                pw = ixpool.tile([128, 128], bf16, tag="pw")
                nc.sync.dma_start(pw[:], pool_d[:, w * 128:(w + 1) * 128])
                nc.tensor.matmul(psg[:], pw[:], h_nm[:, w * H:(w + 1) * H],
                                 start=(w == 0), stop=(w == NW - 1))
            gx = stage.tile([128, H], f32, tag="gx")
            nc.vector.tensor_copy(gx[:], psg[:])
            pst = psB.tile([128, 128], f32, tag="tps")
            nc.tensor.transpose(pst[:H, :], gx[:], ident[:])
            gxT = stage.tile([H + 1, 128], f32, tag="gxT")
            nc.vector.tensor_copy(gxT[:H, :], pst[:H, :])
            nc.vector.memset(gxT[H:H + 1, :], 1.0)
            ps1 = psB.tile([128, H], f32, tag="tps")
            nc.tensor.matmul(ps1[:], gxT[:], wts["L1b"][:], start=True, stop=True)
            r1 = stage.tile([128, H], f32, tag="r1")
            nc.scalar.activation(r1[:], ps1[:], AF.Relu)
            pst2 = psB.tile([128, 128], f32, tag="tps")
            nc.tensor.transpose(pst2[:H, :], r1[:], ident[:])
            r1T = stage.tile([H + 1, 128], f32, tag="r1T")
            nc.vector.tensor_copy(r1T[:H, :], pst2[:H, :])
            nc.vector.memset(r1T[H:H + 1, :], 1.0)
            ps2 = psB.tile([128, 4], f32, tag="tps")
            nc.tensor.matmul(ps2[:, :3], r1T[:], wts["L2b"][:], start=True, stop=True)
            outs = stage.tile([128, 4], f32, tag="outs")
            nc.vector.memset(outs[:], 0.0)
            nc.vector.tensor_copy(outs[:, :3], ps2[:, :3])
            nc.sync.dma_start(out_d[:], outs[:])

    nc.compile()
    return nc


# --------------------------------------------------------------- entry
def prepare_in_maps(inputs, cfg):
    import ml_dtypes
    per_core, meta = shard_and_pack(inputs, cfg)
    w = weights_map(inputs)
    in_maps = []
    for c in range(cfg["nc"]):
        pc = per_core[c]
        bf = ml_dtypes.bfloat16
        m = {}
        for k, v in w.items():
            m[k] = v if k in ("L1b", "L2b", "ident") else v.astype(bf)
        m["gidx"], m["sidx"] = pc["gidx"], pc["sidx"]
        m["iotac"] = pc["iotac"]
        m["ew"] = pc["ew"].astype(bf)
        m["xT1"] = pc["xT1"].astype(bf)
        m["pool"] = pc["pool"].astype(bf)
        in_maps.append(m)
    return in_maps, meta, per_core


def unshard_output(outs_np, out_names, out_avals, per_core, meta):
    NC = meta["cfg"]["nc"]
    i = out_names.index("out")
    arr = outs_np[i].reshape(NC, *out_avals[i].shape)
    outs = [arr[c][:per_core[c]["ngraphs"], :3] for c in range(NC)]
    return np.concatenate(outs, 0).astype(np.float32)


def run(inputs, cfg, trace=False):
    in_maps, meta, per_core = prepare_in_maps(inputs, cfg)
    nc = build_graph(meta)
    from concourse import bass_utils
    res = bass_utils.run_bass_kernel_spmd(
        nc, in_maps, core_ids=list(range(cfg["nc"])), trace=trace)
    outs = [np.asarray(res.results[c]["out"])[:per_core[c]["ngraphs"], :3]
            for c in range(cfg["nc"])]
    return np.concatenate(outs, 0).astype(np.float32), res


def kernel(**inputs):
    out, _ = run(inputs, FULL_CFG)
    return out

